# revision 15
# baseline (speedup 1.0000x reference)
"""Trainium2 Bass kernel for nn_GatedJunction (gated multi-branch junction).

Math (per batch element b):
    m_y  = mean_hw(y[b])                     # [C]
    m_xk = mean_hw(x_k[b])                   # [C] for k=0..3
    feats = concat(m_y, m_x0..m_x3)          # [5C] = [1280]
    h  = relu(bn(feats @ conv1_w.T))         # [32]
    w  = h @ conv2_w.T + conv2_b             # [1280] -> [5, 256]
    w1 = sigmoid(w[0])                       # self gate  [256]
    w2 = softmax_k(w[1:])                    # branch gates [4, 256]
    out[b] = y[b]*w1[:,None,None] + sum_k w2[k][:,None,None]*x_k[b]

Sharding: data-parallel over batch. 8 cores x 4 batch elements each.
Params are tiny, pre-transposed/folded on the host, replicated per core.

v3 design (per-batch pipeline, DMA-bound target ~35us/iter steady,
~45us single-shot):
  - inputs fp16; each batch loads as TWO DMAs ({y,x0,x1} 12.3KB/partition,
    {x2,x3} 8.2KB/partition, both above the ~5KB HW descriptor knee) on
    the SP HWDGE queue, so channel sums of the first piece overlap the
    second piece's transfer. Params are packed into 2 fp32 blobs loaded
    before the first batch (one HWDGE slot each).
  - per-batch chain: channel sums (DVE tensor_scalar accum in 4x fp16
    mode + 2 on ACT) with conv1's accumulating PE matmul emitted right
    after each chunk's sum; BN scale folded into conv1 weights host-side
    and bias+ReLU done as one DVE tensor_scalar (add, max 0); conv2 on
    PE; exp (ACT) before sigmoid so softmax/diag don't wait on it;
    softmax normalize per channel-half (DVE).
  - pass 2 per batch: ch0 = 4 branch diag-matmuls accumulate in PSUM then
    one DVE scalar_tensor_tensor fusing y*w1 + psum -> fp16; ch1 = 5 diag
    matmuls (y included via a diag of w1) then an ACT PSUM->fp16 copy.
    Diag builds run on the otherwise idle Pool engine (DVE for the last
    batch's ch0 to shorten the drain tail). Stores on the ACT HWDGE queue.
  - the gate MLP is per-batch (not batched over 4) and emission is
    software-pipelined one batch deep, so pass 2 of batch b hides under
    the DMA of batch b+1 instead of draining serially after all loads.
"""

import sys

for _p in ("/root/.axon_site/_ro/trn_rl_repo", "/opt/trn_rl_repo"):
    if _p not in sys.path:
        sys.path.append(_p)

from contextlib import ExitStack

import numpy as np

import concourse.bass as bass
import concourse.tile as tile
from concourse import mybir
from concourse.bass_utils import run_bass_kernel_spmd

# Problem constants (hardcoded from the spec).
B, K, C, H, W = 32, 4, 256, 32, 32
MID = 32
EPS = 1e-5
HW = H * W          # 1024
HWH = HW // 2       # 512 (max moving free dim / one PSUM bank of fp32)
N_CORES = 8
B_LOC = B // N_CORES  # 4
NT = K + 1          # 5 tensors: y, x0..x3
FEAT = NT * C       # 1280
NCH = FEAT // 128   # 10 feature chunks of 128
CH = C // 128       # 2 channel chunks per tensor
NTA = 3             # tensors in load piece A (y, x0, x1); piece B: x2, x3

# paramA blob layout ([128, PA_W] fp32): w1T | c2bT | ident
PA_W1, PA_C2B, PA_ID = 0, NCH * MID, NCH * MID + NCH
PA_W = PA_ID + 128
# paramB blob layout ([MID, PB_W] fp32): w2T | bias_eff
PB_W2, PB_BIAS = 0, NCH * 128
PB_W = PB_BIAS + 1

FP32 = mybir.dt.float32
FP16 = mybir.dt.float16
ALU = mybir.AluOpType
AF = mybir.ActivationFunctionType


def _split_waits(nc: bass.Bass) -> None:
    """This toolchain's walrus accepts only ONE sync-wait per instruction
    (setupSyncWait: 'Too many sync wait commands') while Tile emits several.
    Hoist all-but-one wait onto standalone EventSemaphore instructions
    placed immediately before, on the same engine — semantically identical
    (sequencer stalls at each wait in order)."""
    for f in nc.m.functions:
        for blk in f.blocks:
            insts = list(blk.instructions)
            out, changed = [], False
            for inst in insts:
                si = inst.sync_info
                if si is not None and len(si.on_wait) > 1:
                    waits = list(si.on_wait)
                    for i, w in enumerate(waits[:-1]):
                        ev = mybir.InstEventSemaphore(
                            name=f"{inst.name}-sw{i}", ins=[], outs=[]
                        )
                        ev.engine = inst.engine
                        ev.sync_info = mybir.SyncInfo(on_wait=[w], on_update=[])
                        out.append(ev)
                    si.on_wait = [waits[-1]]
                    changed = True
                out.append(inst)
            if changed:
                blk.instructions = out


def build_program(
    repeat: int = 1, loop_reps: bool = False, inner: int = 1, ablate: str = ""
) -> bass.Bass:
    """Emit the single-core SPMD program (same program, per-core data).

    repeat > 1 python-unrolls the whole batch loop (idempotent).
    loop_reps=True instead wraps the batch loop in a hardware For_i whose
    trip count comes from an extra int32 input "reps" — used by test.py to
    time steady-state iterations with launch overhead cancelled exactly.
    inner unrolls the For_i body (inner x batch_seq per trip) to amortize
    the per-trip all-engine barrier / pipeline-drain cost.
    """
    nc = bass.Bass()

    # All 5 feature maps stacked partition-major on the host:
    #   xs[b, p, t, ch, hw] = tensor_t[b, ch*128 + p, hw]
    d_xs = nc.declare_dram_parameter(
        "xs", [B_LOC, 128, NT, CH, HW], FP16, isOutput=False
    )
    # Packed params (see make_in_maps):
    #   paramA[p, :] = w1T (conv1_w pre-transposed, BN scale/HW folded in)
    #                  | c2bT | ident(row p of eye(128), as f32)
    #   paramB[m, :] = w2T (conv2_w pre-transposed) | bias_eff
    d_parA = nc.declare_dram_parameter("paramA", [128, PA_W], FP32, isOutput=False)
    d_parB = nc.declare_dram_parameter("paramB", [MID, PB_W], FP32, isOutput=False)
    # Output partition-major as well: out[b, p, ch, hw] = out_t[b, ch*128+p, hw]
    d_out = nc.declare_dram_parameter("out", [B_LOC, 128, CH, HW], FP16, isOutput=True)
    d_reps = (
        nc.declare_dram_parameter("reps", [1, 1], mybir.dt.int32, isOutput=False)
        if loop_reps
        else None
    )

    with tile.TileContext(nc) as tc, ExitStack() as ctx:
        cpool = ctx.enter_context(tc.tile_pool(name="cpool", bufs=1))
        ppool = ctx.enter_context(tc.tile_pool(name="ppool", bufs=2, space="PSUM"))
        dpool = ctx.enter_context(tc.tile_pool(name="dpool", bufs=2))
        spool = ctx.enter_context(tc.tile_pool(name="spool", bufs=2))

        # ---------------- parameter prep (once) ----------------
        # Two packed param DMAs FIRST on the sync (SP) HWDGE queue (~1.1us),
        # then batch loads follow on the same queue. Matmul stationary
        # tensors are "laundered" through one DVE copy each so PE matmuls
        # (which tolerate only ONE sync-wait on their embedded weight load)
        # depend on a single producer proc (DVE).
        parA = cpool.tile([128, PA_W], FP32, name="parA", tag="parA")
        parB = cpool.tile([MID, PB_W], FP32, name="parB", tag="parB")
        nc.sync.dma_start(out=parA[:], in_=d_parA[:])
        nc.sync.dma_start(out=parB[:], in_=d_parB[:])

        w1T = cpool.tile([128, NCH, MID], FP32, name="w1T", tag="w1T")
        nc.vector.tensor_copy(
            w1T[:], parA[:, PA_W1 : PA_W1 + NCH * MID].rearrange(
                "p (j m) -> p j m", j=NCH
            )
        )
        ident = cpool.tile([128, 128], FP16, name="ident", tag="ident")
        nc.vector.tensor_copy(ident[:], parA[:, PA_ID : PA_ID + 128])
        w2T = cpool.tile([MID, NCH, 128], FP32, name="w2T", tag="w2T")
        nc.vector.tensor_copy(
            w2T[:], parB[:, PB_W2 : PB_W2 + NCH * 128].rearrange(
                "m (j p) -> m j p", j=NCH
            )
        )
        c2bT = parA[:, PA_C2B : PA_C2B + NCH]
        bias_eff = parB[:, PB_BIAS : PB_BIAS + 1]

        def load_batch(b: int):
            dt_ = dpool.tile(
                [128, NT, CH, HW], FP16, name="dt", tag="dt", bufs=7
            )
            nc.sync.dma_start(out=dt_[:, 0:NTA], in_=d_xs[b, :, 0:NTA])
            nc.sync.dma_start(out=dt_[:, NTA:NT], in_=d_xs[b, :, NTA:NT])
            return dt_

        def sums_conv1_batch(b: int, dt_):
            """Channel sums of all 10 chunks (fp32 sums; BN-scale/HW folded
            into w1T host-side), 8 on DVE (4x fp16 mode) + 2 on ACT, with
            conv1's accumulating matmul group consuming each chunk's sum.
            Returns the PSUM accumulator hx [MID, 1]."""
            mean_b = spool.tile(
                [128, NCH], FP32, name="mean_b", tag="mean_b", bufs=3
            )
            scr = spool.tile([128, HW], FP16, name="scr", tag="scr", bufs=2)
            scr_a = spool.tile([128, HW], FP16, name="scr_a", tag="scr_a", bufs=2)
            # ACT takes two chunks of piece A (ready first) so DVE and ACT
            # drain in parallel without lengthening the last-piece tail.
            act_j = (1, 3)
            for j in range(NCH):
                t, ch = divmod(j, CH)
                if j in act_j:
                    nc.scalar.activation(
                        out=scr_a[:],
                        in_=dt_[:, t, ch, :],
                        func=AF.Copy,
                        accum_out=mean_b[:, j : j + 1],
                    )
                else:
                    nc.vector.tensor_scalar(
                        out=scr[:],
                        in0=dt_[:, t, ch, :],
                        scalar1=1.0,
                        scalar2=None,
                        op0=ALU.mult,
                        op1=ALU.add,
                        accum_out=mean_b[:, j : j + 1],
                    )
            hx = ppool.tile([MID, 1], FP32, name="hx", tag="hx", bufs=2)
            for j in range(NCH):
                nc.tensor.matmul(
                    hx[:],
                    w1T[:, j, :],
                    mean_b[:, j : j + 1],
                    start=(j == 0),
                    stop=(j == NCH - 1),
                )
            return hx

        def mlp_batch(b: int, hx):
            """Per-batch gate MLP tail: bias+ReLU (one DVE op) -> conv2
            (PE) -> +bias (DVE) -> exp then sigmoid (ACT; exp first so the
            softmax path doesn't wait) -> softmax normalize over k (DVE)."""
            h_all = spool.tile([MID, 1], FP32, name="h_all", tag="h_all", bufs=2)
            nc.vector.tensor_scalar(
                out=h_all[:],
                in0=hx[:],
                scalar1=bias_eff,
                scalar2=0.0,
                op0=ALU.add,
                op1=ALU.max,
            )
            wps = ppool.tile([128, NCH], FP32, name="wps", tag="wps", bufs=2)
            for j in list(range(CH, NCH)) + list(range(CH)):
                nc.tensor.matmul(
                    wps[:, j : j + 1], w2T[:, j, :], h_all[:], start=True, stop=True
                )
            wlog = spool.tile([128, NCH], FP32, name="wlog", tag="wlog", bufs=2)
            # Branch cols first so exp (and the whole softmax/diag path)
            # doesn't wait on the sigmoid cols.
            nc.vector.tensor_tensor(
                out=wlog[:, CH:NCH], in0=wps[:, CH:NCH], in1=c2bT[:, CH:NCH],
                op=ALU.add,
            )
            gat = spool.tile([128, NCH], FP32, name="gat", tag="gat", bufs=3)
            nc.scalar.activation(
                out=gat[:, CH:NCH], in_=wlog[:, CH:NCH], func=AF.Exp
            )
            nc.vector.tensor_tensor(
                out=wlog[:, 0:CH], in0=wps[:, 0:CH], in1=c2bT[:, 0:CH],
                op=ALU.add,
            )
            nc.scalar.activation(
                out=gat[:, 0:CH], in_=wlog[:, 0:CH], func=AF.Sigmoid
            )
            esum = spool.tile([128, CH, 1], FP32, name="esum", tag="esum", bufs=2)
            rinv = spool.tile([128, CH, 1], FP32, name="rinv", tag="rinv", bufs=3)
            gk = gat[:, CH:NCH].rearrange("p (k c) -> p c k", c=CH)
            nc.vector.reduce_sum(out=esum[:], in_=gk, axis=mybir.AxisListType.X)
            nc.vector.reciprocal(rinv[:], esum[:])
            # The softmax 1/sum normalization is folded into the diag
            # builds (tensor_scalar's second scalar), not applied to gat.
            return gat, rinv

        def pass2_batch(b: int, dt_, gat, rinv, last: bool):
            """Pass 2. ch0: 4 branch diag-matmuls -> PSUM, then one DVE
            scalar_tensor_tensor fuses y*w1 + psum -> fp16 acc. ch1: 5
            diag-matmuls (y via diag of w1) -> PSUM, then ACT copy -> fp16.
            The softmax 1/sum rides along as the diag tensor_scalar's
            second scalar. Diags on Pool; for the last batch they run on
            DVE (Pool's ~300ns/op serial chain would sit on the drain
            tail) and ch1 also takes the sst path (shorter PE chain, no
            ACT copies on the tail)."""
            diag = spool.tile(
                [128, NT * CH, 128], FP16, name="diag", tag="diag", bufs=2
            )
            # diag j -> gate column: branch diags for (t,ch): j = t*CH+ch,
            # t=1..4; y-diag (ch1 only, unless last): j = 1 (gate col ch=1).
            for ch in range(CH):
                ts = range(1, NT) if (ch == 0 or last) else range(NT)
                for t in ts:
                    jj = t * CH + ch
                    eng = nc.vector if last else nc.gpsimd
                    if t == 0:
                        eng.tensor_scalar_mul(
                            out=diag[:, jj, :],
                            in0=ident[:],
                            scalar1=gat[:, jj : jj + 1],
                        )
                    else:
                        eng.tensor_scalar(
                            out=diag[:, jj, :],
                            in0=ident[:],
                            scalar1=gat[:, jj : jj + 1],
                            scalar2=rinv[:, ch, :],
                            op0=ALU.mult,
                            op1=ALU.mult,
                        )
            acc = dpool.tile([128, CH, HW], FP16, name="acc", tag="acc", bufs=3)
            for ch in range(CH):
                sst = ch == 0 or last
                for h2 in range(2):
                    sl = slice(h2 * HWH, (h2 + 1) * HWH)
                    ps = ppool.tile([128, HWH], FP32, name="ps", tag="ps", bufs=3)
                    ts = range(1, NT) if sst else range(NT)
                    for i, t in enumerate(ts):
                        nc.tensor.matmul(
                            ps[:],
                            diag[:, t * CH + ch, :],
                            dt_[:, t, ch, sl],
                            start=(i == 0),
                            stop=(i == len(ts) - 1),
                        )
                    if sst:
                        nc.vector.scalar_tensor_tensor(
                            out=acc[:, ch, sl],
                            in0=dt_[:, 0, ch, sl],
                            scalar=gat[:, ch : ch + 1],
                            in1=ps[:],
                            op0=ALU.mult,
                            op1=ALU.add,
                        )
                    else:
                        nc.scalar.copy(acc[:, ch, sl], ps[:])
            return acc

        def batch_seq():
            if ablate == "dma":  # loads + stores only
                for b in range(B_LOC):
                    dt_ = load_batch(b)
                    nc.scalar.dma_start(out=d_out[b], in_=dt_[:, 0])
                return
            # Software-pipelined emission one batch deep: pass 2 of batch
            # b-1 is emitted between conv1_b and the mlp tail of b, so every
            # in-order engine stream reaches instructions whose dependencies
            # are ~1 batch old (already satisfied).
            pending = None  # (b, dt_, gat, rinv) awaiting pass2 emission
            acc_prev = None  # (b, acc) awaiting store emission
            for b in range(B_LOC):
                dt_ = load_batch(b)
                hx = sums_conv1_batch(b, dt_)
                if pending is not None:
                    acc_prev = (pending[0], pass2_batch(*pending, last=False))
                gat, rinv = mlp_batch(b, hx)
                if acc_prev is not None:
                    # Store of batch b-1 emitted AFTER mlp_b's ACT ops: the
                    # store's DGE setup on the ACT SEQ otherwise delays
                    # exp/sigmoid of batch b by ~1us. Stores go on the ACT
                    # HWDGE queue so they never block the SP load queue.
                    nc.scalar.dma_start(out=d_out[acc_prev[0]], in_=acc_prev[1][:])
                    acc_prev = None
                if ablate == "nopass2":
                    nc.scalar.dma_start(out=d_out[b], in_=dt_[:, 0])
                    continue
                pending = (b, dt_, gat, rinv)
            if pending is not None:
                acc = pass2_batch(*pending, last=True)
                # Tail store on the (now idle) SP queue: its DGE setup
                # doesn't queue behind anything on the ACT SEQ.
                nc.sync.dma_start(out=d_out[pending[0]], in_=acc[:])

        # ---------------- main loop over local batches ----------------
        if loop_reps:
            reps_sb = cpool.tile([1, 1], mybir.dt.int32, name="reps_sb", tag="reps_sb")
            nc.sync.dma_start(out=reps_sb[:], in_=d_reps[:])
            reps_val = nc.values_load(
                reps_sb[0:1, 0:1],
                min_val=1,
                max_val=1_000_000,
                skip_runtime_bounds_check=True,
            )
            with tc.For_i(0, reps_val):
                for _ in range(inner):
                    batch_seq()
        else:
            for _ in range(repeat):
                batch_seq()

    _split_waits(nc)
    return nc


_CACHE: dict = {}


def _get_program() -> bass.Bass:
    if "nc" not in _CACHE:
        _CACHE["nc"] = build_program()
    return _CACHE["nc"]


def make_in_maps(inputs: dict, reps: int | None = None) -> list:
    """Shard full inputs into per-core input maps (batch-parallel)."""
    f32 = lambda a: np.ascontiguousarray(np.asarray(a), dtype=np.float32)
    # Stack the 5 maps partition-major: xs[b, p, t, ch, hw].
    big = np.empty((B, 128, NT, CH, HW), dtype=np.float16)
    for t, nm in enumerate(("y", "x0", "x1", "x2", "x3")):
        a = np.asarray(inputs[nm], dtype=np.float32).astype(np.float16)
        big[:, :, t] = a.reshape(B, CH, 128, HW).transpose(0, 2, 1, 3)

    conv1_w = f32(inputs["conv1_w"])               # [MID, FEAT]
    conv2_w = f32(inputs["conv2_w"])               # [FEAT, MID]
    conv2_b = f32(inputs["conv2_b"])               # [FEAT]
    gamma = f32(inputs["bn_gamma"]).reshape(MID)
    beta = f32(inputs["bn_beta"]).reshape(MID)
    mean = f32(inputs["bn_mean"]).reshape(MID)
    var = f32(inputs["bn_var"]).reshape(MID)

    s_bn = gamma / np.sqrt(var + EPS)
    # w1T[p, j, m] = conv1_w[m, 128j+p] * s_bn[m] / HW  (BN scale + the
    # sums->means normalization folded into the weights).
    w1T = (conv1_w * (s_bn / HW)[:, None]).reshape(MID, NCH, 128).transpose(2, 1, 0)
    parA = np.empty((128, PA_W), dtype=np.float32)
    parA[:, PA_W1 : PA_W1 + NCH * MID] = w1T.reshape(128, NCH * MID)
    parA[:, PA_C2B : PA_C2B + NCH] = conv2_b.reshape(NCH, 128).T
    parA[:, PA_ID : PA_ID + 128] = np.eye(128, dtype=np.float32)
    w2T = conv2_w.reshape(NCH, 128, MID).transpose(2, 0, 1)  # [MID, NCH, 128]
    parB = np.empty((MID, PB_W), dtype=np.float32)
    parB[:, PB_W2 : PB_W2 + NCH * 128] = w2T.reshape(MID, NCH * 128)
    parB[:, PB_BIAS] = beta - mean * s_bn

    shared = {
        "paramA": np.ascontiguousarray(parA),
        "paramB": np.ascontiguousarray(parB),
    }
    if reps is not None:
        shared["reps"] = np.full((1, 1), reps, dtype=np.int32)
    in_maps = []
    for core in range(N_CORES):
        sl = slice(core * B_LOC, (core + 1) * B_LOC)
        m = {"xs": np.ascontiguousarray(big[sl])}
        m.update(shared)
        in_maps.append(m)
    return in_maps


def kernel(**inputs) -> np.ndarray:
    nc = _get_program()
    in_maps = make_in_maps(inputs)
    res = run_bass_kernel_spmd(nc, in_maps, list(range(N_CORES)))
    _CACHE["last_results"] = res
    # Device output is [B_LOC, p, ch, hw]; un-permute to [B_LOC, ch*128+p, h, w].
    out = np.concatenate(
        [
            res.results[i]["out"]
            .transpose(0, 2, 1, 3)
            .reshape(B_LOC, C, H, W)
            for i in range(N_CORES)
        ],
        axis=0,
    )
    return out.astype(np.float32)
